# revision 14
# baseline (speedup 1.0000x reference)
"""BiLinearInteraction Trainium2 kernel (8 NeuronCores, data-parallel over batch).

Reference computation (per pair p=(i,j) of F=26 fields, P=325 pairs):
    out[b, p*64:(p+1)*64] = (x[i, b, :] @ W[p]) * x[j, b, :]
Full shapes: x [26, 4096, 64] f32, W [325, 64, 64] f32 -> out [4096, 20800] f32.

Strategy (v3)
- Shard batch 4096 -> 8 x 512 (4 tiles of 128 rows/core), replicate W.
- HBM traffic ~28 MB/core: out 21.3MB bf16 write + reads ~6.6MB single-copy
  bf16. Even fields' matmul operands (lhsT xt, rhs w) sit in SBUF partitions
  0-63 and odd fields' in 64-127, so PE 2-row-group concurrency
  (tile_position row tiling) needs no duplicated HBM copies; consecutive
  fields' matmul pieces are emitted interleaved to pair the row groups.
- SWDGE descriptor generation costs ~850ns per dma_start serially on the Q7,
  so loads are consolidated to 9 issues: 7 per-chunk w loads (lo/hi packed in
  one [128, cmax] block, pad transferred on the narrow half) + tile-0
  (xn|xt) block + tiles-1-3 block. All input SBUF tiles are one-shot consts.
- Elementwise: measured rates ACT copy 0.833ns/el + 400ns/instr, DVE mul
  0.58ns/el from SBUF bf16 (2x mode, separate dst), 1.3ns/el from PSUM f32.
  Balanced split: the 12 biggest fields drain PSUM->bf16 cp tile on ACT then
  mul on DVE at 2x; the 13 smallest mul straight from PSUM. ~17.3/18.4us per
  tile on ACT/DVE.
- Output staged per (tile, chunk) and written as 7 contiguous bf16 DMAs/tile
  on the SP HWDGE ring; first chunk is one field so writes start early.
"""

import sys

sys.path.insert(0, "/opt/trn_rl_repo")

from itertools import combinations

import ml_dtypes
import numpy as np

import concourse.bass as bass
import concourse.mybir as mybir
from concourse import bacc
from concourse.tile import TileContext

F, D, B = 26, 64, 4096
NCORES = 8
BC = B // NCORES          # 512 batch rows per core
NT = BC // 128            # 4 batch tiles of 128 rows
NF = F - 1                # 25 left fields
PAIRS = list(combinations(range(F), 2))
N_PAIRS = [F - 1 - i for i in range(NF)]            # pairs with left field i
P_START = [sum(N_PAIRS[:i]) for i in range(NF)]     # first pair index of field i
P = sum(N_PAIRS)          # 325
OUT_COLS = P * D          # 20800

# column offset of field i inside the parity-packed w_lo / w_hi streams
WOFF = {}
_ol = _oh = 0
for _i in range(NF):
    if _i % 2 == 0:
        WOFF[_i] = _ol
        _ol += N_PAIRS[_i] * D
    else:
        WOFF[_i] = _oh
        _oh += N_PAIRS[_i] * D

# Output chunks: contiguous field ranges; first/last small for early writes
# and a short tail.
CHUNKS = [(0, 1), (1, 3), (3, 6), (6, 10), (10, 15), (15, 21), (21, 25)]
# per-chunk (first even field, lo cols), (first odd field, hi cols), padded max
CHUNK_LO, CHUNK_HI, CHUNK_MAX = [], [], []
for _f0, _f1 in CHUNKS:
    _ev = [i for i in range(_f0, _f1) if i % 2 == 0]
    _od = [i for i in range(_f0, _f1) if i % 2 == 1]
    _lc = sum(N_PAIRS[i] for i in _ev) * D
    _hc = sum(N_PAIRS[i] for i in _od) * D
    CHUNK_LO.append((_ev[0] if _ev else None, _lc))
    CHUNK_HI.append((_od[0] if _od else None, _hc))
    CHUNK_MAX.append(max(_lc, _hc))
W_PACK_COLS = sum(CHUNK_MAX)

N_DRAIN = 13              # fields 0..12 drained (ACT), 13..24 direct (DVE)
# PE row-group pairs (even field -> partitions 0-63, odd -> 64-127), mostly
# coupling one ACT-drained (big) field with one DVE-direct (small) field so
# ACT and DVE stay loaded simultaneously, ordered so output chunks complete
# progressively (steady write stream, small tail).
PAIR_ORDER = [(0, 1), (14, 13), (2, 15), (16, 3), (4, 17), (18, 5),
              (12, None), (6, 19), (8, 7), (10, 9), (22, 11), (20, 21),
              (24, 23)]
XT_BLK = (len([i for i in range(NF) if i % 2 == 0])) * 128   # 1664 lo cols
XX_TILE = F * D + XT_BLK  # 3328: [xn 1664 | xt 1664] per batch tile

F32 = mybir.dt.float32
BF16 = mybir.dt.bfloat16


def build_bass() -> bass.Bass:
    nc = bacc.Bacc()
    w = nc.declare_dram_parameter("w", [128, W_PACK_COLS], BF16, isOutput=False)
    xt0 = nc.declare_dram_parameter("xt0", [128, XT_BLK], BF16, isOutput=False)
    xn0 = nc.declare_dram_parameter("xn0", [128, F * D], BF16, isOutput=False)
    xx123 = nc.declare_dram_parameter(
        "xx123", [128, 3 * XX_TILE], BF16, isOutput=False)
    out = nc.declare_dram_parameter("out", [BC, OUT_COLS], BF16, isOutput=True)

    with TileContext(nc) as tc:
        with (
            tc.tile_pool(name="consts", bufs=1) as consts,
            tc.tile_pool(name="stage", bufs=2) as stage_pool,
            tc.tile_pool(name="cp_pool", bufs=4) as cp_pool,
            tc.tile_pool(name="psum", bufs=2, space="PSUM") as psum_pool,
        ):
            w_sb = [consts.tile([128, CHUNK_MAX[ci]], BF16,
                                tag=f"w{ci}", name=f"w{ci}")
                    for ci in range(len(CHUNKS))]
            xt0_sb = consts.tile([128, XT_BLK], BF16, tag="xt0", name="xt0")
            xn0_sb = consts.tile([128, F * D], BF16, tag="xn0", name="xn0")
            xx123_sb = consts.tile([128, 3 * XX_TILE], BF16,
                                   tag="xx123", name="xx123")

            # 10 SWDGE loads, just-in-time order: first chunk's weights and
            # tile-0 matmul operand first; tiles 1-3 bulk last.
            _woff = [sum(CHUNK_MAX[:ci]) for ci in range(len(CHUNKS))]
            nc.gpsimd.dma_start(
                out=w_sb[0][:], in_=w[:, _woff[0]:_woff[0] + CHUNK_MAX[0]])
            nc.gpsimd.dma_start(out=xt0_sb[:], in_=xt0[:, :])
            nc.gpsimd.dma_start(out=xn0_sb[:], in_=xn0[:, :])
            for ci in range(1, len(CHUNKS)):
                nc.gpsimd.dma_start(
                    out=w_sb[ci][:], in_=w[:, _woff[ci]:_woff[ci] + CHUNK_MAX[ci]])
            nc.gpsimd.dma_start(out=xx123_sb[:], in_=xx123[:, :])

            field_chunk = {}
            for ci, (f0, f1) in enumerate(CHUNKS):
                for i in range(f0, f1):
                    field_chunk[i] = ci

            def xn_ap(t, c0, c1):
                if t == 0:
                    return xn0_sb[:, c0:c1]
                b = (t - 1) * XX_TILE
                return xx123_sb[:, b + c0:b + c1]

            def xt_ap(t, r0, c0, c1):
                if t == 0:
                    return xt0_sb[r0:r0 + D, c0:c1]
                b = (t - 1) * XX_TILE + F * D
                return xx123_sb[r0:r0 + D, b + c0:b + c1]

            for t in range(NT):
                stage = {}
                remaining = {}
                for ci, (f0, f1) in enumerate(CHUNKS):
                    cols = sum(N_PAIRS[i] for i in range(f0, f1)) * D
                    stage[ci] = stage_pool.tile(
                        [128, cols], BF16, tag=f"st{ci}", name=f"st{t}_{ci}")
                    remaining[ci] = f1 - f0

                def mm_pieces(i):
                    npair = N_PAIRS[i]
                    cols = npair * D
                    g = i % 2
                    r0 = g * D
                    k = i // 2
                    ci = field_chunk[i]
                    lhsT = xt_ap(t, r0, k * 128, (k + 1) * 128)
                    first = CHUNK_LO[ci][0] if g == 0 else CHUNK_HI[ci][0]
                    woff0 = WOFF[i] - WOFF[first]
                    ps = psum_pool.tile([128, cols], F32, tag="ps",
                                        name=f"ps{t}_{i}")
                    pieces = []
                    for s0 in range(0, cols, 512):
                        n = min(512, cols - s0)
                        pieces.append((ps[:, s0:s0 + n], lhsT,
                                       w_sb[ci][r0:r0 + D,
                                                woff0 + s0:woff0 + s0 + n]))
                    return ps, pieces

                def emit_mul(i, src):
                    """DVE mul into the stage tile; fire chunk write when done."""
                    npair = N_PAIRS[i]
                    cols = npair * D
                    ci = field_chunk[i]
                    st = stage[ci]
                    c0 = (P_START[i] - P_START[CHUNKS[ci][0]]) * D
                    xj = xn_ap(t, (i + 1) * D, (i + 1 + npair) * D)
                    nc.vector.tensor_mul(st[:, c0:c0 + cols], src, xj)
                    remaining[ci] -= 1
                    if remaining[ci] == 0:
                        f0, f1 = CHUNKS[ci]
                        cc0 = P_START[f0] * D
                        ccols = sum(N_PAIRS[j] for j in range(f0, f1)) * D
                        nc.sync.dma_start(
                            out=out[t * 128:(t + 1) * 128, cc0:cc0 + ccols],
                            in_=st[:])

                # Software pipeline: per pair, emit matmuls (interleaved for
                # PE row-group concurrency), then ACT drains, then DVE muls
                # for this pair's direct fields, then the PREVIOUS pair's
                # drained-field muls — so DVE never sits behind an ACT drain
                # still in flight.
                pending = []   # [(field, cp_tile)] drained muls not yet emitted
                for fa, fb in PAIR_ORDER:
                    ps_a, pieces_a = mm_pieces(fa)
                    if fb is not None:
                        ps_b, pieces_b = mm_pieces(fb)
                    else:
                        ps_b, pieces_b = None, []
                    for pi in range(max(len(pieces_a), len(pieces_b))):
                        for pieces in (pieces_a, pieces_b):
                            if pi < len(pieces):
                                o, l, r = pieces[pi]
                                nc.tensor.matmul(o, l, r, start=True, stop=True)
                    fields = [(fa, ps_a)] + ([(fb, ps_b)] if fb is not None else [])
                    newly_drained = []
                    for i, ps in fields:
                        if i < N_DRAIN:
                            cp = cp_pool.tile([128, N_PAIRS[i] * D], BF16,
                                              tag="cp", name=f"cp{t}_{i}")
                            nc.scalar.copy(out=cp[:], in_=ps[:])
                            newly_drained.append((i, cp))
                    for i, ps in fields:
                        if i >= N_DRAIN:
                            emit_mul(i, ps[:])
                    for i, cp in pending:
                        emit_mul(i, cp[:])
                    pending = newly_drained
                for i, cp in pending:
                    emit_mul(i, cp[:])
    nc.compile()
    return nc


def prep_inputs(x: np.ndarray, W: np.ndarray):
    """Full inputs -> per-core in_maps with pre-packed bf16 layouts."""
    x = np.ascontiguousarray(np.asarray(x, dtype=np.float32))
    W = np.ascontiguousarray(np.asarray(W, dtype=np.float32))
    # Pair-grouped weights wg[:, p*64+e] = W[p][:, e]; pack per chunk:
    # partitions 0-63 = even (lo) piece, 64-127 = odd (hi) piece, each
    # zero-padded to the chunk's max width.
    wg = W.transpose(1, 0, 2).reshape(D, OUT_COLS)
    wp = np.zeros((128, W_PACK_COLS), dtype=np.float32)
    col = 0
    for ci, (f0, f1) in enumerate(CHUNKS):
        lo = np.concatenate(
            [wg[:, P_START[i] * D:(P_START[i] + N_PAIRS[i]) * D]
             for i in range(f0, f1) if i % 2 == 0], axis=1)
        hi_parts = [wg[:, P_START[i] * D:(P_START[i] + N_PAIRS[i]) * D]
                    for i in range(f0, f1) if i % 2 == 1]
        wp[0:D, col:col + lo.shape[1]] = lo
        if hi_parts:
            hi = np.concatenate(hi_parts, axis=1)
            wp[D:2 * D, col:col + hi.shape[1]] = hi
        col += CHUNK_MAX[ci]
    wp = np.ascontiguousarray(wp.astype(ml_dtypes.bfloat16))

    EV = [i for i in range(NF) if i % 2 == 0]
    OD = [i for i in range(NF) if i % 2 == 1]
    in_maps = []
    for c in range(NCORES):
        xc = x[:, c * BC:(c + 1) * BC, :]                      # [26, 512, 64]
        xr = xc.reshape(F, NT, 128, D)
        xx = np.zeros((NT, 128, XX_TILE), dtype=np.float32)
        for t in range(NT):
            # xn block: [128, 26*64] batch-major field concat
            xx[t, :, :F * D] = xr[:, t].transpose(1, 0, 2).reshape(128, F * D)
            # xt block: [64, 13*128] per parity half (d-major lhsT layout)
            xtl = xr[EV, t].transpose(2, 0, 1).reshape(D, len(EV) * 128)
            xth = xr[OD, t].transpose(2, 0, 1).reshape(D, len(OD) * 128)
            xx[t, 0:D, F * D:F * D + xtl.shape[1]] = xtl
            xx[t, D:2 * D, F * D:F * D + xth.shape[1]] = xth
        xxb = xx.astype(ml_dtypes.bfloat16)
        in_maps.append({
            "w": wp,
            "xt0": np.ascontiguousarray(xxb[0, :, F * D:]),
            "xn0": np.ascontiguousarray(xxb[0, :, :F * D]),
            "xx123": np.ascontiguousarray(
                xxb[1:].transpose(1, 0, 2).reshape(128, 3 * XX_TILE)),
        })
    return in_maps


_CACHED_NC = None


def kernel(x: np.ndarray, W: np.ndarray) -> np.ndarray:
    global _CACHED_NC
    from concourse.bass_utils import run_bass_kernel_spmd

    if _CACHED_NC is None:
        _CACHED_NC = build_bass()
    in_maps = prep_inputs(x, W)
    res = run_bass_kernel_spmd(_CACHED_NC, in_maps, list(range(NCORES)))
    shards = [
        np.asarray(res.results[c]["out"]).astype(np.float32) for c in range(NCORES)
    ]
    return np.concatenate(shards, axis=0)


# revision 19
# speedup vs baseline: 1.2848x; 1.2848x over previous
"""BiLinearInteraction Trainium2 kernel (8 NeuronCores, data-parallel over batch).

Reference computation (per pair p=(i,j) of F=26 fields, P=325 pairs):
    out[b, p*64:(p+1)*64] = (x[i, b, :] @ W[p]) * x[j, b, :]
Full shapes: x [26, 4096, 64] f32, W [325, 64, 64] f32 -> out [4096, 20800] f32.

Strategy (v3)
- Shard batch 4096 -> 8 x 512 (4 tiles of 128 rows/core), replicate W.
- HBM traffic ~28 MB/core: out 21.3MB bf16 write + reads ~6.6MB single-copy
  bf16. Even fields' matmul operands (lhsT xt, rhs w) sit in SBUF partitions
  0-63 and odd fields' in 64-127, so PE 2-row-group concurrency
  (tile_position row tiling) needs no duplicated HBM copies; consecutive
  fields' matmul pieces are emitted interleaved to pair the row groups.
- SWDGE descriptor generation costs ~850ns per dma_start serially on the Q7,
  so loads are consolidated to 9 issues: 7 per-chunk w loads (lo/hi packed in
  one [128, cmax] block, pad transferred on the narrow half) + tile-0
  (xn|xt) block + tiles-1-3 block. All input SBUF tiles are one-shot consts.
- Elementwise: measured rates ACT copy 0.833ns/el + 400ns/instr, DVE mul
  0.58ns/el from SBUF bf16 (2x mode, separate dst), 1.3ns/el from PSUM f32.
  Balanced split: the 12 biggest fields drain PSUM->bf16 cp tile on ACT then
  mul on DVE at 2x; the 13 smallest mul straight from PSUM. ~17.3/18.4us per
  tile on ACT/DVE.
- Output staged per (tile, chunk) and written as 7 contiguous bf16 DMAs/tile
  on the SP HWDGE ring; first chunk is one field so writes start early.
"""

import sys

sys.path.insert(0, "/opt/trn_rl_repo")

from itertools import combinations

import ml_dtypes
import numpy as np

import concourse.bass as bass
import concourse.mybir as mybir
from concourse import bacc
from concourse.tile import TileContext

F, D, B = 26, 64, 4096
NCORES = 8
BC = B // NCORES          # 512 batch rows per core
NT = BC // 128            # 4 batch tiles of 128 rows
NF = F - 1                # 25 left fields
PAIRS = list(combinations(range(F), 2))
N_PAIRS = [F - 1 - i for i in range(NF)]            # pairs with left field i
P_START = [sum(N_PAIRS[:i]) for i in range(NF)]     # first pair index of field i
P = sum(N_PAIRS)          # 325
OUT_COLS = P * D          # 20800

# column offset of field i inside the parity-packed w_lo / w_hi streams
WOFF = {}
_ol = _oh = 0
for _i in range(NF):
    if _i % 2 == 0:
        WOFF[_i] = _ol
        _ol += N_PAIRS[_i] * D
    else:
        WOFF[_i] = _oh
        _oh += N_PAIRS[_i] * D

# Output chunks: contiguous field ranges; first/last small for early writes
# and a short tail.
CHUNKS = [(0, 1), (1, 3), (3, 6), (6, 10), (10, 15), (15, 21), (21, 25)]
# per-chunk (first even field, lo cols), (first odd field, hi cols), padded max
CHUNK_LO, CHUNK_HI, CHUNK_MAX = [], [], []
for _f0, _f1 in CHUNKS:
    _ev = [i for i in range(_f0, _f1) if i % 2 == 0]
    _od = [i for i in range(_f0, _f1) if i % 2 == 1]
    _lc = sum(N_PAIRS[i] for i in _ev) * D
    _hc = sum(N_PAIRS[i] for i in _od) * D
    CHUNK_LO.append((_ev[0] if _ev else None, _lc))
    CHUNK_HI.append((_od[0] if _od else None, _hc))
    CHUNK_MAX.append(max(_lc, _hc))
W_PACK_COLS = sum(CHUNK_MAX)

N_DRAIN = 13              # fields 0..12 drained (ACT), 13..24 direct (DVE)
# Fields 0-8 are processed as two half-width units so the largest PSUM tile
# is 1024 f32 = 2 banks and the PSUM pool fits FOUR buffers (bank-granular
# allocation, 4x2 = 8 banks). With only two buffers every pair holds both
# and the PE serializes against the consumers (~30us of measured PE stalls).
SPLIT = {0: 800, 1: 768, 2: 736, 3: 704, 4: 672, 5: 640, 6: 608, 7: 576,
         8: 544}                           # half width (cols) for split fields
# units: (field, col offset, cols)
UNITS = {}
for _i in range(NF):
    _c = N_PAIRS[_i] * D
    if _i in SPLIT:
        _h = SPLIT[_i]
        UNITS[(_i, 0)] = (_i, 0, _h)
        UNITS[(_i, 1)] = (_i, _h, _c - _h)
    else:
        UNITS[(_i, 0)] = (_i, 0, _c)
# PE row-group pairs of units (even field -> partitions 0-63, odd -> 64-127),
# mixing ACT-drained and DVE-direct units so both engines stay loaded,
# ordered so output chunks complete progressively (steady write stream,
# small tail: the last chunks to finish are the 0.72MB and 0.16MB ones).
PAIR_ORDER = [((0, 0), (13, 0)), ((0, 1), (1, 0)), ((14, 0), (1, 1)),
              ((2, 0), (15, 0)), ((2, 1), (3, 0)), ((16, 0), (3, 1)),
              ((4, 0), (17, 0)), ((4, 1), (5, 0)), ((18, 0), (5, 1)),
              ((6, 0), (7, 0)), ((6, 1), (7, 1)), ((8, 0), (9, 0)),
              ((8, 1), (11, 0)), ((10, 0), (19, 0)), ((12, 0), (21, 0)),
              ((20, 0), (23, 0)), ((22, 0), None), ((24, 0), None)]
XT_BLK = (len([i for i in range(NF) if i % 2 == 0])) * 128   # 1664 lo cols
XX_TILE = F * D + XT_BLK  # 3328: [xn 1664 | xt 1664] per batch tile

F32 = mybir.dt.float32
BF16 = mybir.dt.bfloat16


def build_bass() -> bass.Bass:
    nc = bacc.Bacc()
    w = nc.declare_dram_parameter("w", [128, W_PACK_COLS], BF16, isOutput=False)
    xt0 = nc.declare_dram_parameter("xt0", [128, XT_BLK], BF16, isOutput=False)
    xn0 = nc.declare_dram_parameter("xn0", [128, F * D], BF16, isOutput=False)
    xx123 = nc.declare_dram_parameter(
        "xx123", [128, 3 * XX_TILE], BF16, isOutput=False)
    out = nc.declare_dram_parameter("out", [BC, OUT_COLS], BF16, isOutput=True)

    with TileContext(nc) as tc:
        with (
            tc.tile_pool(name="consts", bufs=1) as consts,
            tc.tile_pool(name="stage", bufs=2) as stage_pool,
            tc.tile_pool(name="cp_pool", bufs=4) as cp_pool,
            tc.tile_pool(name="psum", bufs=4, space="PSUM") as psum_pool,
        ):
            w_sb = [consts.tile([128, CHUNK_MAX[ci]], BF16,
                                tag=f"w{ci}", name=f"w{ci}")
                    for ci in range(len(CHUNKS))]
            xt0_sb = consts.tile([128, XT_BLK], BF16, tag="xt0", name="xt0")
            xn0_sb = consts.tile([128, F * D], BF16, tag="xn0", name="xn0")
            xx123_sb = consts.tile([128, 3 * XX_TILE], BF16,
                                   tag="xx123", name="xx123")

            # 10 SWDGE loads, just-in-time order: first chunk's weights and
            # tile-0 matmul operand first; tiles 1-3 bulk last.
            _woff = [sum(CHUNK_MAX[:ci]) for ci in range(len(CHUNKS))]
            nc.gpsimd.dma_start(
                out=w_sb[0][:], in_=w[:, _woff[0]:_woff[0] + CHUNK_MAX[0]])
            nc.gpsimd.dma_start(out=xt0_sb[:], in_=xt0[:, :])
            nc.gpsimd.dma_start(out=xn0_sb[:], in_=xn0[:, :])
            for ci in range(1, len(CHUNKS)):
                nc.gpsimd.dma_start(
                    out=w_sb[ci][:], in_=w[:, _woff[ci]:_woff[ci] + CHUNK_MAX[ci]])
            nc.gpsimd.dma_start(out=xx123_sb[:], in_=xx123[:, :])

            field_chunk = {}
            for ci, (f0, f1) in enumerate(CHUNKS):
                for i in range(f0, f1):
                    field_chunk[i] = ci

            def xn_ap(t, c0, c1):
                if t == 0:
                    return xn0_sb[:, c0:c1]
                b = (t - 1) * XX_TILE
                return xx123_sb[:, b + c0:b + c1]

            def xt_ap(t, r0, c0, c1):
                if t == 0:
                    return xt0_sb[r0:r0 + D, c0:c1]
                b = (t - 1) * XX_TILE + F * D
                return xx123_sb[r0:r0 + D, b + c0:b + c1]

            for t in range(NT):
                stage = {}
                remaining = {}
                for ci, (f0, f1) in enumerate(CHUNKS):
                    cols = sum(N_PAIRS[i] for i in range(f0, f1)) * D
                    stage[ci] = stage_pool.tile(
                        [128, cols], BF16, tag=f"st{ci}", name=f"st{t}_{ci}")
                    remaining[ci] = sum(
                        2 if i in SPLIT else 1 for i in range(f0, f1))

                def mm_pieces(u):
                    i, off, cols = UNITS[u]
                    g = i % 2
                    r0 = g * D
                    k = i // 2
                    ci = field_chunk[i]
                    lhsT = xt_ap(t, r0, k * 128, (k + 1) * 128)
                    first = CHUNK_LO[ci][0] if g == 0 else CHUNK_HI[ci][0]
                    woff0 = WOFF[i] - WOFF[first] + off
                    ps = psum_pool.tile([128, cols], F32, tag="ps",
                                        name=f"ps{t}_{i}_{off}")
                    pieces = []
                    for s0 in range(0, cols, 512):
                        n = min(512, cols - s0)
                        pieces.append((ps[:, s0:s0 + n], lhsT,
                                       w_sb[ci][r0:r0 + D,
                                                woff0 + s0:woff0 + s0 + n]))
                    return ps, pieces

                def emit_mul(u, src):
                    """DVE mul into the stage tile; fire chunk write when done."""
                    i, off, cols = UNITS[u]
                    ci = field_chunk[i]
                    st = stage[ci]
                    c0 = (P_START[i] - P_START[CHUNKS[ci][0]]) * D + off
                    xj = xn_ap(t, (i + 1) * D + off, (i + 1) * D + off + cols)
                    nc.vector.tensor_mul(st[:, c0:c0 + cols], src, xj)
                    remaining[ci] -= 1
                    if remaining[ci] == 0:
                        f0, f1 = CHUNKS[ci]
                        cc0 = P_START[f0] * D
                        ccols = sum(N_PAIRS[j] for j in range(f0, f1)) * D
                        nc.sync.dma_start(
                            out=out[t * 128:(t + 1) * 128, cc0:cc0 + ccols],
                            in_=st[:])

                # Software pipeline: per pair, emit matmuls (interleaved for
                # PE row-group concurrency), then ACT drains, then DVE muls
                # for this pair's direct units, then the PREVIOUS pair's
                # drained-unit muls — so DVE never sits behind an ACT drain
                # still in flight.
                pending = []   # [(unit, cp_tile)] drained muls not yet emitted
                for ua, ub in PAIR_ORDER:
                    ps_a, pieces_a = mm_pieces(ua)
                    if ub is not None:
                        ps_b, pieces_b = mm_pieces(ub)
                    else:
                        ps_b, pieces_b = None, []
                    for pi in range(max(len(pieces_a), len(pieces_b))):
                        for pieces in (pieces_a, pieces_b):
                            if pi < len(pieces):
                                o, l, r = pieces[pi]
                                nc.tensor.matmul(o, l, r, start=True, stop=True)
                    units = [(ua, ps_a)] + ([(ub, ps_b)] if ub is not None else [])
                    newly_drained = []
                    for u, ps in units:
                        if u[0] < N_DRAIN:
                            cp = cp_pool.tile([128, UNITS[u][2]], BF16,
                                              tag="cp", name=f"cp{t}_{u[0]}_{u[1]}")
                            nc.scalar.copy(out=cp[:], in_=ps[:])
                            newly_drained.append((u, cp))
                    for u, ps in units:
                        if u[0] >= N_DRAIN:
                            emit_mul(u, ps[:])
                    for u, cp in pending:
                        emit_mul(u, cp[:])
                    pending = newly_drained
                for u, cp in pending:
                    emit_mul(u, cp[:])
    nc.compile()
    return nc


def prep_inputs(x: np.ndarray, W: np.ndarray):
    """Full inputs -> per-core in_maps with pre-packed bf16 layouts."""
    x = np.ascontiguousarray(np.asarray(x, dtype=np.float32))
    W = np.ascontiguousarray(np.asarray(W, dtype=np.float32))
    # Pair-grouped weights wg[:, p*64+e] = W[p][:, e]; pack per chunk:
    # partitions 0-63 = even (lo) piece, 64-127 = odd (hi) piece, each
    # zero-padded to the chunk's max width.
    wg = W.transpose(1, 0, 2).reshape(D, OUT_COLS)
    wp = np.zeros((128, W_PACK_COLS), dtype=np.float32)
    col = 0
    for ci, (f0, f1) in enumerate(CHUNKS):
        lo = np.concatenate(
            [wg[:, P_START[i] * D:(P_START[i] + N_PAIRS[i]) * D]
             for i in range(f0, f1) if i % 2 == 0], axis=1)
        hi_parts = [wg[:, P_START[i] * D:(P_START[i] + N_PAIRS[i]) * D]
                    for i in range(f0, f1) if i % 2 == 1]
        wp[0:D, col:col + lo.shape[1]] = lo
        if hi_parts:
            hi = np.concatenate(hi_parts, axis=1)
            wp[D:2 * D, col:col + hi.shape[1]] = hi
        col += CHUNK_MAX[ci]
    wp = np.ascontiguousarray(wp.astype(ml_dtypes.bfloat16))

    EV = [i for i in range(NF) if i % 2 == 0]
    OD = [i for i in range(NF) if i % 2 == 1]
    in_maps = []
    for c in range(NCORES):
        xc = x[:, c * BC:(c + 1) * BC, :]                      # [26, 512, 64]
        xr = xc.reshape(F, NT, 128, D)
        xx = np.zeros((NT, 128, XX_TILE), dtype=np.float32)
        for t in range(NT):
            # xn block: [128, 26*64] batch-major field concat
            xx[t, :, :F * D] = xr[:, t].transpose(1, 0, 2).reshape(128, F * D)
            # xt block: [64, 13*128] per parity half (d-major lhsT layout)
            xtl = xr[EV, t].transpose(2, 0, 1).reshape(D, len(EV) * 128)
            xth = xr[OD, t].transpose(2, 0, 1).reshape(D, len(OD) * 128)
            xx[t, 0:D, F * D:F * D + xtl.shape[1]] = xtl
            xx[t, D:2 * D, F * D:F * D + xth.shape[1]] = xth
        xxb = xx.astype(ml_dtypes.bfloat16)
        in_maps.append({
            "w": wp,
            "xt0": np.ascontiguousarray(xxb[0, :, F * D:]),
            "xn0": np.ascontiguousarray(xxb[0, :, :F * D]),
            "xx123": np.ascontiguousarray(
                xxb[1:].transpose(1, 0, 2).reshape(128, 3 * XX_TILE)),
        })
    return in_maps


_CACHED_NC = None


def kernel(x: np.ndarray, W: np.ndarray) -> np.ndarray:
    global _CACHED_NC
    from concourse.bass_utils import run_bass_kernel_spmd

    if _CACHED_NC is None:
        _CACHED_NC = build_bass()
    in_maps = prep_inputs(x, W)
    res = run_bass_kernel_spmd(_CACHED_NC, in_maps, list(range(NCORES)))
    shards = [
        np.asarray(res.results[c]["out"]).astype(np.float32) for c in range(NCORES)
    ]
    return np.concatenate(shards, axis=0)
